# revision 27
# baseline (speedup 1.0000x reference)
"""Trainium2 Bass kernel for nn_DeChunkLayer (ragged EMA de-chunk).

Math (per batch row b):
    p[l]   = clip(boundary_prob[b, l, 1], EPS, 1-EPS)
    nb[l]  = cumsum_l(boundary_mask[b])          (>= 1 since l=0 is a boundary)
    h(k)   = (1-p_s[k]) h(k-1) + p_s[k] x[k]     (EMA over chunk index k,
                                                  p_s = p at the k-th boundary)
    out[l] = h(nb[l]-1)

Collapses to ONE first-order scan in l-space:
    out[l] = a[l]*out[l-1] + bvec[l]
    a[l]   = 1 - pm[l],  pm[l] = boundary_mask[l]*p[l]
    bvec[l]= pm[l] * x[nb[l]-1]
At a boundary l with chunk rank k = nb[l]-1 this performs exactly the EMA
step h(k) = (1-p[l]) h(k-1) + p[l] x[k] (the same pairing the reference's
argsort produces); at non-boundary positions a=1, bvec=0 holds the value.

Device plan (per core; core = (b, dhalf), D_shard = 512):
  1. preprocessing (all small tiles): clip p, pm = mask*p, nb = cumsum
     (within-column cumsum via triangular matmul + column offsets via a
     tiny free-dim scan), idx = nb-1 as int32 in col-major layout
     ([128, 64]: column j = the 128 indices for l-tile j).
  2. per 128-l tile: indirect DMA gathers xg = x[idx[l]] (128 rows x
     2KB; offsets MUST be [128,1] — one per partition — batched offset
     tables silently stream contiguous rows on HW).
  3. bn = pm_col * xg (DVE tensor_scalar, 2x mode), then PE transpose
     (transpose mode, fp32-exact) into [128_d, 512_l] PSUM tiles.
  4. DVE tensor_tensor_scan (state = a*state + b) along l per d-tile;
     a = (1-pm) broadcast to 128 partitions via a ones-matmul; carry
     [128,1] tiles chain the 16 chunks.
  5. PE-transpose back to [128_l, 512_d], ACT copies into one
     [128, 2048] staging tile per chunk, single HWDGE DMA out per chunk
     (1 MiB contiguous).
Measured on TRN2: ~150 us/core (mem roofline ~94 us); engine busy:
PE ~101 us (512 transposes), DVE ~105 us (scan 76 + tensor_scalar 29),
Pool ~72 us (64 SWDGE indirect-descriptor gens), ACT ~53 us.

kernel(**inputs) takes FULL inputs, shards over 8 cores (4 batch rows x 2
D-halves), returns FULL (4, 8192, 1024) f32 output.
"""

import os
import sys

import numpy as np

sys.path.insert(0, "/opt/trn_rl_repo")

B, L, D = 4, 8192, 1024
NCORES = 8
DSH = D // 2          # 512 channels per core
NLT = L // 128        # 64 l-tiles of 128
NCH = L // 512        # 16 chunks of 512
NDT = DSH // 128      # 4 d-tiles of 128
EPS = 1e-4

_prog = None  # cached compiled Bass program


def _build_program(reps=1, mode='full'):
    import concourse.bass as bass
    import concourse.mybir as mybir
    from concourse import bacc
    from concourse.bass import IndirectOffsetOnAxis
    from concourse.masks import make_identity, make_upper_triangular
    from concourse.tile import TileContext

    f32 = mybir.dt.float32
    i32 = mybir.dt.int32
    u8 = mybir.dt.uint8
    Op = mybir.AluOpType

    nc = bacc.Bacc("TRN2", target_bir_lowering=False, debug=False,
                   num_devices=NCORES)

    x = nc.declare_dram_parameter("x", [L, DSH], f32, isOutput=False)
    bp = nc.declare_dram_parameter("bp", [64, 256], f32, isOutput=False)
    bm = nc.declare_dram_parameter("bm", [64, 128], u8, isOutput=False)
    out = nc.declare_dram_parameter("out", [L, DSH], f32, isOutput=True)

    with TileContext(nc) as tc:
        with (
            tc.tile_pool(name="const", bufs=1) as cpool,
            tc.tile_pool(name="prep", bufs=1) as ppool,
        ):
            # ---- constants ----
            ident = cpool.tile([128, 128], f32, tag="ident")
            make_identity(nc, ident[:])
            ut1 = cpool.tile([128, 128], f32, tag="ut1")
            make_upper_triangular(nc, ut1[:], 1.0, diag=True)
            ones1 = cpool.tile([1, 128], f32, tag="ones1")
            nc.vector.memset(ones1[:], 1.0)
            ones_col = cpool.tile([128, 1], f32, tag="ones_col")
            nc.vector.memset(ones_col[:], 1.0)
            zeros1 = cpool.tile([1, 64], f32, tag="zeros1")
            nc.vector.memset(zeros1[:], 0.0)

            # ---- scalar preprocessing ----
            # row-major [64,128]: element [p, f] = l = 128*p + f
            bm_u8 = ppool.tile([64, 128], u8, tag="bm_u8")
            nc.sync.dma_start(out=bm_u8[:], in_=bm[:])
            bmf = ppool.tile([64, 128], f32, tag="bmf")
            nc.vector.tensor_copy(bmf[:], bm_u8[:])

            bp_rm = ppool.tile([64, 256], f32, tag="bp_rm")
            nc.sync.dma_start(out=bp_rm[:], in_=bp[:])
            p_rm = ppool.tile([64, 128], f32, tag="p_rm")
            nc.vector.tensor_scalar(
                out=p_rm[:], in0=bp_rm[:][:, 1::2],
                scalar1=EPS, scalar2=1.0 - EPS, op0=Op.max, op1=Op.min)
            pm_rm = ppool.tile([64, 128], f32, tag="pm_rm")
            nc.vector.tensor_tensor(
                out=pm_rm[:], in0=p_rm[:], in1=bmf[:], op=Op.mult)
            arow_rm = ppool.tile([64, 128], f32, tag="arow_rm")
            nc.vector.tensor_scalar(
                out=arow_rm[:], in0=pm_rm[:],
                scalar1=-1.0, scalar2=1.0, op0=Op.mult, op1=Op.add)
            # [1, 8192] rows on partition 0 (base-partition-0 slices for
            # per-chunk broadcast matmuls)
            arow1 = ppool.tile([1, L], f32, tag="arow1")
            nc.sync.dma_start(out=arow1[:], in_=arow_rm[:])

            bm_cm = ppool.tile([128, 64], f32, tag="bm_cm")
            pm_cm = ppool.tile([128, 64], f32, tag="pm_cm")
            colsum = ppool.tile([1, 64], f32, tag="colsum")
            csum = ppool.tile([1, 64], f32, tag="csum")
            excl = ppool.tile([1, 64], f32, tag="excl")
            idxf = ppool.tile([128, 64], f32, tag="idxf")
            idx = ppool.tile([128, 64], i32, tag="idx")

            with tc.tile_pool(name="pps", bufs=1, space="PSUM") as pps:
                # col-major [128,64]: element [q, g] = l = q + 128*g
                bmT_ps = pps.tile([128, 64], f32, tag="bmT")
                nc.tensor.transpose(out=bmT_ps[:], in_=bmf[:],
                                    identity=ident[:][:64, :64])
                nc.vector.tensor_copy(bm_cm[:], bmT_ps[:])
                pmT_ps = pps.tile([128, 64], f32, tag="pmT")
                nc.tensor.transpose(out=pmT_ps[:], in_=pm_rm[:],
                                    identity=ident[:][:64, :64])
                nc.vector.tensor_copy(pm_cm[:], pmT_ps[:])

                # nb = within-column inclusive cumsum + per-column offsets
                nb_ps = pps.tile([128, 64], f32, tag="nb")
                nc.tensor.matmul(out=nb_ps[:], lhsT=ut1[:], rhs=bm_cm[:],
                                 start=True, stop=False)
                cs_ps = pps.tile([1, 64], f32, tag="cs")
                nc.tensor.matmul(out=cs_ps[:], lhsT=ones_col[:], rhs=bm_cm[:],
                                 start=True, stop=True)
                nc.vector.tensor_copy(colsum[:], cs_ps[:])
                nc.vector.tensor_tensor_scan(
                    out=csum[:], data0=colsum[:], data1=zeros1[:],
                    initial=0.0, op0=Op.add, op1=Op.add)
                nc.vector.tensor_tensor(
                    out=excl[:], in0=csum[:], in1=colsum[:], op=Op.subtract)
                nc.tensor.matmul(out=nb_ps[:], lhsT=ones1[:], rhs=excl[:],
                                 start=False, stop=True)

                # idx = max(nb-1, 0), col-major (col j = l-tile j)
                nc.vector.tensor_scalar(
                    out=idxf[:], in0=nb_ps[:],
                    scalar1=1.0, scalar2=0.0, op0=Op.subtract, op1=Op.max)
                nc.vector.tensor_copy(idx[:], idxf[:])

            # persistent per-d-tile scan carries
            carries = [ppool.tile([128, 1], f32, tag=f"carry{t}",
                                  name=f"carry{t}")
                       for t in range(NDT)]

            # ---- main loop over 16 chunks of 512 positions ----
            with (
                tc.tile_pool(name="xg", bufs=6) as xgp,
                tc.tile_pool(name="bn", bufs=8) as bnp,
                tc.tile_pool(name="bt", bufs=6, space="PSUM") as btp,
                tc.tile_pool(name="apm", bufs=1, space="PSUM") as apmp,
                tc.tile_pool(name="asb", bufs=4) as asbp,
                tc.tile_pool(name="outT", bufs=4) as otp,
                tc.tile_pool(name="po", bufs=1, space="PSUM") as pop,
                tc.tile_pool(name="ost", bufs=4) as ostp,
            ):
                def front(c, rep):
                    # a_sb = broadcast of (1-pm)[chunk] to 128 partitions
                    apm = apmp.tile([128, 512], f32, tag="apm",
                                    name=f"apm_{c}_{rep}")
                    nc.tensor.matmul(
                        out=apm[:], lhsT=ones1[:],
                        rhs=arow1[:][0:1, 512 * c:512 * (c + 1)],
                        start=True, stop=True)
                    a_sb = asbp.tile([128, 512], f32, tag="a_sb",
                                     name=f"asb_{c}_{rep}")
                    nc.scalar.copy(out=a_sb[:], in_=apm[:])

                    # NOTE: offsets must be [128, 1] — HW consumes one
                    # offset per partition (batched [128,k] offset tables
                    # stream contiguous rows on HW, unlike CoreSim).
                    xg4 = xgp.tile([128, 4 * 512], f32, tag="xg",
                                   name=f"xg_{c}_{rep}")
                    for jj in range(4):
                        nc.gpsimd.indirect_dma_start(
                            out=xg4[:][:, 512 * jj:512 * (jj + 1)],
                            out_offset=None, in_=x[:],
                            in_offset=IndirectOffsetOnAxis(
                                ap=idx[:][:, 4 * c + jj:4 * c + jj + 1],
                                axis=0))

                    bts = [btp.tile([128, 512], f32, tag="bt",
                                    name=f"bt{t}_{c}_{rep}")
                           for t in range(NDT)]
                    for jj in range(4):
                        j = 4 * c + jj
                        bn = bnp.tile([128, 512], f32, tag="bn",
                                      name=f"bn_{c}_{jj}_{rep}")
                        nc.vector.tensor_scalar_mul(
                            bn[:], xg4[:][:, 512 * jj:512 * (jj + 1)],
                            pm_cm[:][:, j:j + 1])
                        for t in range(NDT):
                            nc.tensor.transpose(
                                out=bts[t][:][:, 128 * jj:128 * (jj + 1)],
                                in_=bn[:][:, 128 * t:128 * (t + 1)],
                                identity=ident[:])

                    outTs = [otp.tile([128, 512], f32, tag=f"outT{t}",
                                      name=f"outT{t}_{c}_{rep}")
                             for t in range(NDT)]
                    for t in range(NDT):
                        nc.vector.tensor_tensor_scan(
                            out=outTs[t][:], data0=a_sb[:], data1=bts[t][:],
                            initial=(0.0 if (c == 0 and rep == 0)
                                     else carries[t][:][:, 0:1]),
                            op0=Op.mult, op1=Op.add)
                        nc.vector.tensor_copy(carries[t][:][:, 0:1],
                                              outTs[t][:][:, 511:512])
                    return outTs

                def back(c, rep, outTs):
                    ost = ostp.tile([128, 2048], f32, tag="ost",
                                    name=f"ost_{c}_{rep}")
                    for jj in range(4):
                        po = pop.tile([128, 512], f32, tag="po",
                                      name=f"po_{c}_{jj}_{rep}")
                        for t in range(NDT):
                            nc.tensor.transpose(
                                out=po[:][:, 128 * t:128 * (t + 1)],
                                in_=outTs[t][:][:, 128 * jj:128 * (jj + 1)],
                                identity=ident[:])
                        nc.scalar.copy(out=ost[:][:, 512 * jj:512 * (jj + 1)],
                                       in_=po[:])
                    nc.sync.dma_start(
                        out=out[:][512 * c:512 * (c + 1), :].rearrange(
                            "(b a) d -> a b d", a=128),
                        in_=ost[:].rearrange("a (b d) -> a b d", b=4))

                # software-pipelined emission: front of chunk c+1 is
                # emitted (and thus scheduler-prioritized) before the
                # output side of chunk c, keeping the serial DVE scan
                # chain fed.
                for rep in range(reps):
                    pend = []
                    for c in range(NCH):
                        pend.append((c, front(c, rep)))
                        if len(pend) > 2:
                            pc, po_ = pend.pop(0)
                            back(pc, rep, po_)
                    for pc, po_ in pend:
                        back(pc, rep, po_)

    nc.compile()
    return nc



def _install_profile_hook():
    """Provide antenv.axon_hooks (missing in this image) so
    run_bass_kernel_spmd(trace=True) can capture NTFF profiles via
    /opt/axon/libaxon_pjrt.so."""
    import sys as _sys
    import types
    import contextlib
    import ctypes

    if "antenv.axon_hooks" in _sys.modules:
        return
    try:
        lib = ctypes.CDLL("/opt/axon/libaxon_pjrt.so")
        if not hasattr(lib, "axon_start_nrt_profile"):
            return
    except OSError:
        return
    lib.axon_start_nrt_profile.argtypes = [
        ctypes.POINTER(ctypes.c_int64), ctypes.c_size_t]
    lib.axon_start_nrt_profile.restype = ctypes.c_int64
    lib.axon_stop_nrt_profile.argtypes = [ctypes.c_char_p]
    lib.axon_stop_nrt_profile.restype = ctypes.c_int64

    @contextlib.contextmanager
    def _hook(output_dir, device_ids):
        import jax
        jax.devices()
        if device_ids:
            ids = (ctypes.c_int64 * len(device_ids))(*device_ids)
            rc = lib.axon_start_nrt_profile(ids, len(device_ids))
        else:
            rc = lib.axon_start_nrt_profile(None, 0)
        if rc != 0:
            raise RuntimeError(f"axon_start_nrt_profile rc={rc}")
        try:
            yield
        finally:
            n = lib.axon_stop_nrt_profile(str(output_dir).encode())
            print(f"profile: {n} file(s) written to {output_dir}",
                  file=sys.stderr)

    m = types.ModuleType("antenv.axon_hooks")
    m.get_axon_ntff_profile_hook = lambda: _hook
    m.set_axon_ntff_profile_hook = lambda h: None
    _sys.modules["antenv.axon_hooks"] = m


def _get_program():
    global _prog
    if _prog is None:
        _prog = _build_program()
    return _prog


def run(inputs, trace=False):
    """Returns (full_output, exec_time_ns or None)."""
    from concourse.bass_utils import run_bass_kernel_spmd

    hidden_states = np.asarray(inputs["hidden_states"], dtype=np.float32)
    boundary_mask = np.asarray(inputs["boundary_mask"])
    boundary_prob = np.asarray(inputs["boundary_prob"], dtype=np.float32)

    nc = _get_program()
    in_maps = []
    for c in range(NCORES):
        b, h = divmod(c, 2)
        in_maps.append({
            "x": np.ascontiguousarray(hidden_states[b, :, h * DSH:(h + 1) * DSH]),
            "bp": np.ascontiguousarray(boundary_prob[b].reshape(64, 256)),
            "bm": np.ascontiguousarray(
                boundary_mask[b].astype(np.uint8).reshape(64, 128)),
        })
    if trace:
        _install_profile_hook()
    res = run_bass_kernel_spmd(nc, in_maps, list(range(NCORES)), trace=trace)
    outs = res.results
    full = np.empty((B, L, D), np.float32)
    for c in range(NCORES):
        b, h = divmod(c, 2)
        full[b, :, h * DSH:(h + 1) * DSH] = outs[c]["out"]
    return full, res.exec_time_ns


def kernel(**inputs) -> np.ndarray:
    out, _ = run(inputs, trace=False)
    return out


# revision 28
# speedup vs baseline: 1.3302x; 1.3302x over previous
"""Trainium2 Bass kernel for nn_DeChunkLayer (ragged EMA de-chunk).

Math (per batch row b):
    p[l]   = clip(boundary_prob[b, l, 1], EPS, 1-EPS)
    nb[l]  = cumsum_l(boundary_mask[b])          (>= 1 since l=0 is a boundary)
    h(k)   = (1-p_s[k]) h(k-1) + p_s[k] x[k]     (EMA over chunk index k,
                                                  p_s = p at the k-th boundary)
    out[l] = h(nb[l]-1)

Collapses to ONE first-order scan in l-space:
    out[l] = a[l]*out[l-1] + bvec[l]
    a[l]   = 1 - pm[l],  pm[l] = boundary_mask[l]*p[l]
    bvec[l]= pm[l] * x[nb[l]-1]
At a boundary l with chunk rank k = nb[l]-1 this performs exactly the EMA
step h(k) = (1-p[l]) h(k-1) + p[l] x[k] (the same pairing the reference's
argsort produces); at non-boundary positions a=1, bvec=0 holds the value.

Device plan (per core; core = (b, dhalf), D_shard = 512):
  1. preprocessing (all small tiles): clip p, pm = mask*p, nb = cumsum
     (within-column cumsum via triangular matmul + column offsets via a
     tiny free-dim scan), idx = nb-1 as int32 in col-major layout
     ([128, 64]: column j = the 128 indices for l-tile j).
  2. per 128-l tile: indirect DMA gathers xg = x[idx[l]] (128 rows x
     2KB; offsets MUST be [128,1] — one per partition — batched offset
     tables silently stream contiguous rows on HW).
  3. bn = pm_col * xg (DVE tensor_scalar, 2x mode), then PE transpose
     (transpose mode, fp32-exact) into [128_d, 512_l] PSUM tiles.
  4. DVE tensor_tensor_scan (state = a*state + b) along l per d-tile;
     a = (1-pm) broadcast to 128 partitions via a ones-matmul; carry
     [128,1] tiles chain the 16 chunks.
  5. PE-transpose back to [128_l, 512_d], ACT copies into one
     [128, 2048] staging tile per chunk, single HWDGE DMA out per chunk
     (1 MiB contiguous).
Measured on TRN2: ~150 us/core (mem roofline ~94 us); engine busy:
PE ~101 us (512 transposes), DVE ~105 us (scan 76 + tensor_scalar 29),
Pool ~72 us (64 SWDGE indirect-descriptor gens), ACT ~53 us.

kernel(**inputs) takes FULL inputs, shards over 8 cores (4 batch rows x 2
D-halves), returns FULL (4, 8192, 1024) f32 output.
"""

import os
import sys

import numpy as np

sys.path.insert(0, "/opt/trn_rl_repo")

B, L, D = 4, 8192, 1024
NCORES = 8
DSH = D // 2          # 512 channels per core
NLT = L // 128        # 64 l-tiles of 128
NCH = L // 512        # 16 chunks of 512
NDT = DSH // 128      # 4 d-tiles of 128
EPS = 1e-4

_prog = None  # cached compiled Bass program


def _build_program(reps=1, mode='full'):
    import concourse.bass as bass
    import concourse.mybir as mybir
    from concourse import bacc
    from concourse.bass import IndirectOffsetOnAxis
    from concourse.masks import make_identity, make_upper_triangular
    from concourse.tile import TileContext

    f32 = mybir.dt.float32
    i32 = mybir.dt.int32
    u8 = mybir.dt.uint8
    Op = mybir.AluOpType

    nc = bacc.Bacc("TRN2", target_bir_lowering=False, debug=False,
                   num_devices=NCORES)

    x = nc.declare_dram_parameter("x", [L, DSH], f32, isOutput=False)
    bp = nc.declare_dram_parameter("bp", [64, 256], f32, isOutput=False)
    bm = nc.declare_dram_parameter("bm", [64, 128], u8, isOutput=False)
    out = nc.declare_dram_parameter("out", [L, DSH], f32, isOutput=True)

    with TileContext(nc) as tc:
        with (
            tc.tile_pool(name="const", bufs=1) as cpool,
            tc.tile_pool(name="prep", bufs=1) as ppool,
        ):
            # ---- constants ----
            ident = cpool.tile([128, 128], f32, tag="ident")
            make_identity(nc, ident[:])
            ut1 = cpool.tile([128, 128], f32, tag="ut1")
            make_upper_triangular(nc, ut1[:], 1.0, diag=True)
            ones1 = cpool.tile([1, 128], f32, tag="ones1")
            nc.vector.memset(ones1[:], 1.0)
            ones_col = cpool.tile([128, 1], f32, tag="ones_col")
            nc.vector.memset(ones_col[:], 1.0)
            zeros1 = cpool.tile([1, 64], f32, tag="zeros1")
            nc.vector.memset(zeros1[:], 0.0)

            # ---- scalar preprocessing ----
            # row-major [64,128]: element [p, f] = l = 128*p + f
            bm_u8 = ppool.tile([64, 128], u8, tag="bm_u8")
            nc.sync.dma_start(out=bm_u8[:], in_=bm[:])
            bmf = ppool.tile([64, 128], f32, tag="bmf")
            nc.vector.tensor_copy(bmf[:], bm_u8[:])

            bp_rm = ppool.tile([64, 256], f32, tag="bp_rm")
            nc.sync.dma_start(out=bp_rm[:], in_=bp[:])
            p_rm = ppool.tile([64, 128], f32, tag="p_rm")
            nc.vector.tensor_scalar(
                out=p_rm[:], in0=bp_rm[:][:, 1::2],
                scalar1=EPS, scalar2=1.0 - EPS, op0=Op.max, op1=Op.min)
            pm_rm = ppool.tile([64, 128], f32, tag="pm_rm")
            nc.vector.tensor_tensor(
                out=pm_rm[:], in0=p_rm[:], in1=bmf[:], op=Op.mult)
            arow_rm = ppool.tile([64, 128], f32, tag="arow_rm")
            nc.vector.tensor_scalar(
                out=arow_rm[:], in0=pm_rm[:],
                scalar1=-1.0, scalar2=1.0, op0=Op.mult, op1=Op.add)
            # [1, 8192] rows on partition 0 (base-partition-0 slices for
            # per-chunk broadcast matmuls)
            arow1 = ppool.tile([1, L], f32, tag="arow1")
            nc.sync.dma_start(out=arow1[:], in_=arow_rm[:])

            bm_cm = ppool.tile([128, 64], f32, tag="bm_cm")
            pm_cm = ppool.tile([128, 64], f32, tag="pm_cm")
            colsum = ppool.tile([1, 64], f32, tag="colsum")
            csum = ppool.tile([1, 64], f32, tag="csum")
            excl = ppool.tile([1, 64], f32, tag="excl")
            idxf = ppool.tile([128, 64], f32, tag="idxf")
            idx = ppool.tile([128, 64], i32, tag="idx")

            with tc.tile_pool(name="pps", bufs=1, space="PSUM") as pps:
                # col-major [128,64]: element [q, g] = l = q + 128*g
                bmT_ps = pps.tile([128, 64], f32, tag="bmT")
                nc.tensor.transpose(out=bmT_ps[:], in_=bmf[:],
                                    identity=ident[:][:64, :64])
                nc.vector.tensor_copy(bm_cm[:], bmT_ps[:])
                pmT_ps = pps.tile([128, 64], f32, tag="pmT")
                nc.tensor.transpose(out=pmT_ps[:], in_=pm_rm[:],
                                    identity=ident[:][:64, :64])
                nc.vector.tensor_copy(pm_cm[:], pmT_ps[:])

                # nb = within-column inclusive cumsum + per-column offsets
                nb_ps = pps.tile([128, 64], f32, tag="nb")
                nc.tensor.matmul(out=nb_ps[:], lhsT=ut1[:], rhs=bm_cm[:],
                                 start=True, stop=False)
                cs_ps = pps.tile([1, 64], f32, tag="cs")
                nc.tensor.matmul(out=cs_ps[:], lhsT=ones_col[:], rhs=bm_cm[:],
                                 start=True, stop=True)
                nc.vector.tensor_copy(colsum[:], cs_ps[:])
                nc.vector.tensor_tensor_scan(
                    out=csum[:], data0=colsum[:], data1=zeros1[:],
                    initial=0.0, op0=Op.add, op1=Op.add)
                nc.vector.tensor_tensor(
                    out=excl[:], in0=csum[:], in1=colsum[:], op=Op.subtract)
                nc.tensor.matmul(out=nb_ps[:], lhsT=ones1[:], rhs=excl[:],
                                 start=False, stop=True)

                # idx = max(nb-1, 0), col-major (col j = l-tile j)
                nc.vector.tensor_scalar(
                    out=idxf[:], in0=nb_ps[:],
                    scalar1=1.0, scalar2=0.0, op0=Op.subtract, op1=Op.max)
                nc.vector.tensor_copy(idx[:], idxf[:])

            # persistent per-d-tile scan carries
            carries = [ppool.tile([128, 1], f32, tag=f"carry{t}",
                                  name=f"carry{t}")
                       for t in range(NDT)]

            # ---- main loop over 16 chunks of 512 positions ----
            with (
                tc.tile_pool(name="xg", bufs=6) as xgp,
                tc.tile_pool(name="bn", bufs=8) as bnp,
                tc.tile_pool(name="bt", bufs=5, space="PSUM") as btp,
                tc.tile_pool(name="apm", bufs=1, space="PSUM") as apmp,
                tc.tile_pool(name="asb", bufs=4) as asbp,
                tc.tile_pool(name="outT", bufs=4) as otp,
                tc.tile_pool(name="po", bufs=2, space="PSUM") as pop,
                tc.tile_pool(name="ost", bufs=4) as ostp,
            ):
                def front(c, rep):
                    # a_sb = broadcast of (1-pm)[chunk] to 128 partitions
                    apm = apmp.tile([128, 512], f32, tag="apm",
                                    name=f"apm_{c}_{rep}")
                    nc.tensor.matmul(
                        out=apm[:], lhsT=ones1[:],
                        rhs=arow1[:][0:1, 512 * c:512 * (c + 1)],
                        start=True, stop=True)
                    a_sb = asbp.tile([128, 512], f32, tag="a_sb",
                                     name=f"asb_{c}_{rep}")
                    nc.scalar.copy(out=a_sb[:], in_=apm[:])

                    # NOTE: offsets must be [128, 1] — HW consumes one
                    # offset per partition (batched [128,k] offset tables
                    # stream contiguous rows on HW, unlike CoreSim).
                    xg4 = xgp.tile([128, 4 * 512], f32, tag="xg",
                                   name=f"xg_{c}_{rep}")
                    for jj in range(4):
                        nc.gpsimd.indirect_dma_start(
                            out=xg4[:][:, 512 * jj:512 * (jj + 1)],
                            out_offset=None, in_=x[:],
                            in_offset=IndirectOffsetOnAxis(
                                ap=idx[:][:, 4 * c + jj:4 * c + jj + 1],
                                axis=0))

                    bts = [btp.tile([128, 512], f32, tag="bt",
                                    name=f"bt{t}_{c}_{rep}")
                           for t in range(NDT)]
                    for jj in range(4):
                        j = 4 * c + jj
                        bn = bnp.tile([128, 512], f32, tag="bn",
                                      name=f"bn_{c}_{jj}_{rep}")
                        nc.vector.tensor_scalar_mul(
                            bn[:], xg4[:][:, 512 * jj:512 * (jj + 1)],
                            pm_cm[:][:, j:j + 1])
                        for t in range(NDT):
                            nc.tensor.transpose(
                                out=bts[t][:][:, 128 * jj:128 * (jj + 1)],
                                in_=bn[:][:, 128 * t:128 * (t + 1)],
                                identity=ident[:])

                    outTs = [otp.tile([128, 512], f32, tag=f"outT{t}",
                                      name=f"outT{t}_{c}_{rep}")
                             for t in range(NDT)]
                    for t in range(NDT):
                        nc.vector.tensor_tensor_scan(
                            out=outTs[t][:], data0=a_sb[:], data1=bts[t][:],
                            initial=(0.0 if (c == 0 and rep == 0)
                                     else carries[t][:][:, 0:1]),
                            op0=Op.mult, op1=Op.add)
                        nc.vector.tensor_copy(carries[t][:][:, 0:1],
                                              outTs[t][:][:, 511:512])
                    return outTs

                def back(c, rep, outTs):
                    ost = ostp.tile([128, 2048], f32, tag="ost",
                                    name=f"ost_{c}_{rep}")
                    for jj in range(4):
                        po = pop.tile([128, 512], f32, tag="po",
                                      name=f"po_{c}_{jj}_{rep}")
                        for t in range(NDT):
                            nc.tensor.transpose(
                                out=po[:][:, 128 * t:128 * (t + 1)],
                                in_=outTs[t][:][:, 128 * jj:128 * (jj + 1)],
                                identity=ident[:])
                        nc.scalar.copy(out=ost[:][:, 512 * jj:512 * (jj + 1)],
                                       in_=po[:])
                    nc.sync.dma_start(
                        out=out[:][512 * c:512 * (c + 1), :].rearrange(
                            "(b a) d -> a b d", a=128),
                        in_=ost[:].rearrange("a (b d) -> a b d", b=4))

                # software-pipelined emission: front of chunk c+1 is
                # emitted (and thus scheduler-prioritized) before the
                # output side of chunk c, keeping the serial DVE scan
                # chain fed.
                for rep in range(reps):
                    prev = None
                    for c in range(NCH):
                        outTs = front(c, rep)
                        if prev is not None:
                            back(prev[0], rep, prev[1])
                        prev = (c, outTs)
                    back(prev[0], rep, prev[1])

    nc.compile()
    return nc



def _install_profile_hook():
    """Provide antenv.axon_hooks (missing in this image) so
    run_bass_kernel_spmd(trace=True) can capture NTFF profiles via
    /opt/axon/libaxon_pjrt.so."""
    import sys as _sys
    import types
    import contextlib
    import ctypes

    if "antenv.axon_hooks" in _sys.modules:
        return
    try:
        lib = ctypes.CDLL("/opt/axon/libaxon_pjrt.so")
        if not hasattr(lib, "axon_start_nrt_profile"):
            return
    except OSError:
        return
    lib.axon_start_nrt_profile.argtypes = [
        ctypes.POINTER(ctypes.c_int64), ctypes.c_size_t]
    lib.axon_start_nrt_profile.restype = ctypes.c_int64
    lib.axon_stop_nrt_profile.argtypes = [ctypes.c_char_p]
    lib.axon_stop_nrt_profile.restype = ctypes.c_int64

    @contextlib.contextmanager
    def _hook(output_dir, device_ids):
        import jax
        jax.devices()
        if device_ids:
            ids = (ctypes.c_int64 * len(device_ids))(*device_ids)
            rc = lib.axon_start_nrt_profile(ids, len(device_ids))
        else:
            rc = lib.axon_start_nrt_profile(None, 0)
        if rc != 0:
            raise RuntimeError(f"axon_start_nrt_profile rc={rc}")
        try:
            yield
        finally:
            n = lib.axon_stop_nrt_profile(str(output_dir).encode())
            print(f"profile: {n} file(s) written to {output_dir}",
                  file=sys.stderr)

    m = types.ModuleType("antenv.axon_hooks")
    m.get_axon_ntff_profile_hook = lambda: _hook
    m.set_axon_ntff_profile_hook = lambda h: None
    _sys.modules["antenv.axon_hooks"] = m


def _get_program():
    global _prog
    if _prog is None:
        _prog = _build_program()
    return _prog


def run(inputs, trace=False):
    """Returns (full_output, exec_time_ns or None)."""
    from concourse.bass_utils import run_bass_kernel_spmd

    hidden_states = np.asarray(inputs["hidden_states"], dtype=np.float32)
    boundary_mask = np.asarray(inputs["boundary_mask"])
    boundary_prob = np.asarray(inputs["boundary_prob"], dtype=np.float32)

    nc = _get_program()
    in_maps = []
    for c in range(NCORES):
        b, h = divmod(c, 2)
        in_maps.append({
            "x": np.ascontiguousarray(hidden_states[b, :, h * DSH:(h + 1) * DSH]),
            "bp": np.ascontiguousarray(boundary_prob[b].reshape(64, 256)),
            "bm": np.ascontiguousarray(
                boundary_mask[b].astype(np.uint8).reshape(64, 128)),
        })
    if trace:
        _install_profile_hook()
    res = run_bass_kernel_spmd(nc, in_maps, list(range(NCORES)), trace=trace)
    outs = res.results
    full = np.empty((B, L, D), np.float32)
    for c in range(NCORES):
        b, h = divmod(c, 2)
        full[b, :, h * DSH:(h + 1) * DSH] = outs[c]["out"]
    return full, res.exec_time_ns


def kernel(**inputs) -> np.ndarray:
    out, _ = run(inputs, trace=False)
    return out
